# revision 8
# baseline (speedup 1.0000x reference)
"""ALiBi causal multi-head attention on 8 TRN2 NeuronCores.

Problem: x[2,2048,1024] -> qkv proj (16 heads, d=64) -> ALiBi-biased causal
softmax attention -> out proj [1024,1024] + bias.

Sharding: core = (batch b in {0,1}) x (head quad g in {0..3}).  Core (b,g)
holds heads {g, 4+g, 8+g, 12+g} -- one head from each ALiBi slope TIER, so
the distance-cut program (below) is identical across cores (SPMD) while the
head assignment varies only through the input data.  Host sums the 4 quad
partials per batch and adds b_out.

Key tricks (on top of the baseline's layout tricks):
  - ALiBi distance cut: head slot s only attends keys with
    slope_s*(distance) <= ~18 (plus all diagonal tiles).  Dropped tiles
    contribute < ~1e-6 relative mass.  Slot tier cut distances D are sized
    for the SHALLOWEST slope in the tier: slot0 D=72 (slope>=0.25),
    slot1 D=288 (>=2^-4), slot2 D=1152 (>=2^-6), slot3 uncut.
    Removes ~24% of sim/av matmuls and exp work.
  - Diagonal causal mask applied AFTER exp via gpsimd.affine_select
    (replace-with-0, so exp overflow to inf in the dead triangle is
    harmless), instead of a -30000 DVE add on PSUM before exp.
  - Attention emitted tile-major across the 4 head slots with lag-1 AV,
    and projection/out-proj matmul chains for LATER chunks interleaved as
    "filler" PE work between sim and av, hiding the Scalar exp latency.
  - Softmax denominator from a ones-column in V (row 64 of AV output);
    reciprocal_approx_fast reads it straight from PSUM; the broadcast
    multiply uses a partition-stride-0 AP on the DVE.
  - All inputs packed host-side into a few large DMA-friendly tensors
    (xT chunk-major so chunk-0 compute can start after ~1.5MB lands).
  - sim is computed transposed (simT [keys, queries]); ALiBi folded into
    the matmul via two aug contraction rows; qT/kT padded to 96 partitions
    (K<=64 matmuls run at half rate on TRN2).
"""

import sys

for _p in ("/opt/trn_rl_repo", "/root/.axon_site/_ro/trn_rl_repo"):
    if _p not in sys.path:
        sys.path.append(_p)

import numpy as np
from math import log2, floor

import concourse.bass as bass
import concourse.mybir as mybir
import concourse.tile as tile
from concourse import bacc, bass_utils

F32 = mybir.dt.float32
F16 = mybir.dt.float16
AF = mybir.ActivationFunctionType
ALU = mybir.AluOpType

B = 2          # batches
NH = 16        # total heads
H = 4          # heads (slots) per core
D = 64         # head dim
N = 2048       # sequence length
DM = 1024      # model dim
CH = 512       # query chunk
NCH = N // CH  # 4
KD = DM // 128 # 8 contraction tiles for projections
SCALE = D ** -0.5
N_WARMUP = 20
DVE_BCAST = False  # DVE rejects partition-stride-0 APs; use gpsimd broadcast

# First included key tile per (slot, chunk):
# t_start = max(0, ceil((512c - 127 - D_slot)/128)), D = 72/288/1152/inf
TSTART = [
    [0, 3, 7, 11],   # slot 0: slope >= 0.25
    [0, 1, 5, 9],    # slot 1: slope >= 2^-4
    [0, 0, 0, 3],    # slot 2: slope >= 2^-6
    [0, 0, 0, 0],    # slot 3: uncut
]


def _slopes(heads):
    def pow2_slopes(n):
        start = 2 ** (-(2 ** (-(log2(n) - 3))))
        return [start * (start ** i) for i in range(n)]
    if log2(heads).is_integer():
        return pow2_slopes(heads)
    c = 2 ** floor(log2(heads))
    return pow2_slopes(c) + pow2_slopes(2 * c)[0::2][: heads - c]


def build_program():
    nc = bacc.Bacc("TRN2", target_bir_lowering=False, debug=False, num_devices=8)
    xtp = nc.dram_tensor("xtp", [128, NCH * KD * CH], F16, kind="ExternalInput").ap()
    wqp = nc.dram_tensor("wqp", [128, KD * 256], F16, kind="ExternalInput").ap()
    wkp = nc.dram_tensor("wkp", [128, KD * 256], F16, kind="ExternalInput").ap()
    wvp = nc.dram_tensor("wvp", [128, KD * 256], F16, kind="ExternalInput").ap()
    wop = nc.dram_tensor("wop", [128, 2 * DM], F16, kind="ExternalInput").ap()
    qaug = nc.dram_tensor("qaug", [H, 2, N], F16, kind="ExternalInput").ap()
    kaug = nc.dram_tensor("kaug", [2, N], F16, kind="ExternalInput").ap()
    out = nc.dram_tensor("out", [N, DM], F16, kind="ExternalOutput").ap()

    with tile.TileContext(nc) as tc:
        with tc.tile_pool(name="persist", bufs=1) as cp:
            # ---- persistent tiles ----
            xts = cp.tile([128, NCH * KD * CH], F16, tag="xts", name="xts")
            wvs = cp.tile([128, KD * 256], F16, tag="wvs", name="wvs")
            wqs = cp.tile([128, KD * 256], F16, tag="wqs", name="wqs")
            wks = cp.tile([128, KD * 256], F16, tag="wks", name="wks")
            wos = cp.tile([128, 2 * DM], F16, tag="wos", name="wos")
            qt = [cp.tile([96, N], F16, tag=f"qt{s}", name=f"qt{s}")
                  for s in range(H)]
            kt = [cp.tile([96, N], F16, tag=f"kt{s}", name=f"kt{s}")
                  for s in range(H)]
            vsb = [cp.tile([128, H, 65], F16, tag=f"v{r}", name=f"v{r}")
                   for r in range(N // 128)]
            avt = [cp.tile([128, N], F16, tag=f"avt{p}", name=f"avt{p}")
                   for p in range(2)]
            warm = cp.tile([128, CH], F16, tag="warm", name="warm")
            expw = cp.tile([1, 8], F16, tag="expw", name="expw")

            # ---- DMAs, chunk-0-first ----
            def xdma(c):
                nc.sync.dma_start(xts[:, 4096 * c:4096 * (c + 1)],
                                  xtp[:, 4096 * c:4096 * (c + 1)])
            # warm tile + Exp table preload first so the PE warm-up and the
            # ACT table load run during the DMA head.
            nc.vector.memset(warm[:], 0.0)
            nc.scalar.activation(expw[:], warm[0:1, 0:8], AF.Exp)

            nc.sync.dma_start(wvs[:], wvp[:])
            xdma(0)
            nc.sync.dma_start(wqs[:], wqp[:])
            nc.sync.dma_start(wks[:], wkp[:])
            xdma(1)
            # zero the 96-pad rows BEFORE the aug DMAs (WAW ordering makes
            # the DMA land on top of rows 64:66), then stream the aug rows.
            # gpsimd runs these in parallel with the DMA head.
            for s in range(H):
                nc.gpsimd.memset(qt[s][64:96, :], 0.0)
                nc.gpsimd.memset(kt[s][64:96, :], 0.0)
            for s in range(H):
                nc.sync.dma_start(qt[s][64:66, :], qaug[s])
            for s in range(H):
                nc.sync.dma_start(kt[s][64:66, :], kaug[:])
            nc.sync.dma_start(wos[:], wop[:])
            xdma(2)
            xdma(3)

            # ones-columns of V
            for r in range(N // 128):
                nc.gpsimd.memset(vsb[r][:, :, 64], 1.0)

            with tc.tile_pool(name="psc", bufs=2, space="PSUM") as psc, \
                 tc.tile_pool(name="pssim", bufs=2, space="PSUM") as pss, \
                 tc.tile_pool(name="psav", bufs=1, space="PSUM") as psa, \
                 tc.tile_pool(name="ptp", bufs=8) as ptp, \
                 tc.tile_pool(name="smsb", bufs=3) as smsb, \
                 tc.tile_pool(name="osb", bufs=3) as osb:

                # PE warm-up: keeps the PE busy (HAM clock ramp) while the
                # first DMAs land.
                ps_w = psc.tile([128, CH], F32, tag="chain", name="ps_warm")
                for i in range(N_WARMUP):
                    nc.tensor.matmul(ps_w[:], warm[:, 0:128], warm[:],
                                     start=True, stop=True)

                # ---- filler generators: each yield = one PE matmul ----
                def vproj_gen(c):
                    for r in range(4 * c, 4 * c + 4):
                        ps = psc.tile([128, CH], F32, tag="chain",
                                      name=f"psv{r}")
                        base = 4096 * c + 128 * (r % 4)
                        for k in range(KD):
                            nc.tensor.matmul(
                                ps[:, 0:H * D],
                                xts[:, base + 512 * k:base + 512 * k + 128],
                                wvs[:, 256 * k:256 * (k + 1)],
                                start=(k == 0), stop=(k == KD - 1))
                            if k < KD - 1:
                                yield
                        for h in range(H):
                            nc.vector.tensor_copy(
                                vsb[r][:, h, 0:64],
                                ps[:, 64 * h:64 * (h + 1)])
                        yield

                def qkproj_gen(c):
                    for hp in range(2):
                        for wsrc, dst in ((wqs, qt), (wks, kt)):
                            ps = psc.tile([128, CH], F32, tag="chain",
                                          name=f"psqk{hp}_{c}")
                            for k in range(KD):
                                nc.tensor.matmul(
                                    ps[:],
                                    wsrc[:, 256 * k + 128 * hp:
                                         256 * k + 128 * (hp + 1)],
                                    xts[:, 4096 * c + 512 * k:
                                        4096 * c + 512 * (k + 1)],
                                    start=(k == 0), stop=(k == KD - 1))
                                if k < KD - 1:
                                    yield
                            nc.vector.tensor_copy(
                                dst[2 * hp][0:64, CH * c:CH * (c + 1)],
                                ps[0:64, :])
                            nc.vector.tensor_copy(
                                dst[2 * hp + 1][0:64, CH * c:CH * (c + 1)],
                                ps[64:128, :])
                            yield

                def outproj_gen(c):
                    for u in range(4 * c, 4 * c + 4):
                        o_sb = osb.tile([128, DM], F16, tag="osb",
                                        name=f"osb{u}")
                        for nchk in range(2):
                            ps = psc.tile([128, CH], F32, tag="chain",
                                          name=f"psout{u}_{nchk}")
                            for kk in range(2):
                                nc.tensor.matmul(
                                    ps[:],
                                    avt[kk][:, 128 * u:128 * (u + 1)],
                                    wos[:, DM * kk + CH * nchk:
                                        DM * kk + CH * (nchk + 1)],
                                    start=(kk == 0), stop=(kk == 1))
                                if kk == 0:
                                    yield
                            if c == NCH - 1 and nchk == 1:
                                nc.scalar.activation(
                                    o_sb[:, CH * nchk:CH * (nchk + 1)],
                                    ps[:], AF.Copy)
                            else:
                                nc.vector.tensor_copy(
                                    o_sb[:, CH * nchk:CH * (nchk + 1)], ps[:])
                            yield
                        nc.sync.dma_start(out[128 * u:128 * (u + 1), :],
                                          o_sb[:])

                # Ordered filler queue. Force-drain markers ensure the data
                # needed by attention chunk c is emitted before that chunk.
                gens = []
                for c in range(1, NCH):
                    gens.append((f"qk{c}", qkproj_gen(c)))
                    gens.append((f"v{c}", vproj_gen(c)))
                    if c >= 2:
                        gens.append((f"out{c - 2}", outproj_gen(c - 2)))
                gens.append((f"out{NCH - 2}", outproj_gen(NCH - 2)))
                gen_pos = [0]

                def pop_filler():
                    while gen_pos[0] < len(gens):
                        try:
                            next(gens[gen_pos[0]][1])
                            return True
                        except StopIteration:
                            gen_pos[0] += 1
                    return False

                def drain_through(tag):
                    idx = next(i for i, (t, _) in enumerate(gens) if t == tag)
                    while gen_pos[0] <= idx:
                        if not pop_filler():
                            break

                def emit_sim_exp(s, c, t):
                    sd = t - 4 * c
                    lo = 128 * sd if sd >= 0 else 0
                    ps_s = pss.tile([128, CH], F32, tag="pssim",
                                    name=f"pssim{s}_{c}_{t}")
                    nc.tensor.matmul(
                        ps_s[:, lo:CH],
                        kt[s][0:96, 128 * t:128 * (t + 1)],
                        qt[s][0:96, CH * c + lo:CH * (c + 1)],
                        start=True, stop=True)
                    pt_t = ptp.tile([128, CH], F16, tag="pt",
                                    name=f"pt{s}_{c}_{t}")
                    nc.scalar.activation(pt_t[:, lo:CH], ps_s[:, lo:CH],
                                         AF.Exp)
                    if sd >= 0:
                        # causal triangle: keep where (query - key) >= 0
                        nc.gpsimd.affine_select(
                            out=pt_t[:, lo:lo + 128],
                            in_=pt_t[:, lo:lo + 128],
                            compare_op=ALU.is_ge,
                            fill=0.0,
                            base=0,
                            pattern=[[1, 128]],
                            channel_multiplier=-1)
                    return pt_t, lo

                def emit_av(s, c, t, ps_av, pt_t, lo):
                    nc.tensor.matmul(
                        ps_av[:, lo:CH],
                        vsb[t][:, s, :],
                        pt_t[:, lo:CH],
                        start=(t == TSTART[s][c]), stop=(t == 4 * c + 3))

                def emit_normalize(s, c, ps_av):
                    # custom-DVE recip can't source PSUM on HW; bounce the
                    # denominator row through SBUF with a plain DVE copy.
                    dn = smsb.tile([1, CH], F32, tag="dn", name=f"dn{s}_{c}")
                    nc.vector.tensor_copy(dn[:], ps_av[64:65, :])
                    rc = smsb.tile([1, CH], F32, tag="rc", name=f"rc{s}_{c}")
                    nc.vector.reciprocal_approx_fast(rc[:], dn[:])
                    dst = avt[s // 2][64 * (s % 2):64 * (s % 2) + 64,
                                      CH * c:CH * (c + 1)]
                    if DVE_BCAST:
                        nc.vector.tensor_mul(dst, ps_av[0:64, :],
                                             rc.partition_broadcast(64))
                    else:
                        rcb = smsb.tile([D, CH], F32, tag="rcb",
                                        name=f"rcb{s}_{c}")
                        nc.gpsimd.partition_broadcast(rcb[:], rc[:])
                        nc.vector.tensor_mul(dst, ps_av[0:64, :], rcb[:])

                # chunk 0 projections run plainly under the warmup/DMA head
                for _ in vproj_gen(0):
                    pass
                for _ in qkproj_gen(0):
                    pass

                for c in range(NCH):
                    if c > 0:
                        drain_through(f"qk{c}")
                        drain_through(f"v{c}")
                    ps_avs = [psa.tile([65, CH], F32, tag=f"psav{s}",
                                       name=f"psav{s}_{c}")
                              for s in range(H)]
                    pending = {}
                    for t in range(min(TSTART[s][c] for s in range(H)),
                                   4 * c + 4):
                        for s in range(H):
                            if t < TSTART[s][c]:
                                continue
                            pt_t, lo = emit_sim_exp(s, c, t)
                            pop_filler()
                            if s in pending:
                                emit_av(s, c, *pending.pop(s))
                            pending[s] = (t, ps_avs[s], pt_t, lo)
                    for s in range(H):
                        emit_av(s, c, *pending.pop(s))
                        emit_normalize(s, c, ps_avs[s])

                # drain leftovers, then the last chunk's out-projection
                while pop_filler():
                    pass
                for _ in outproj_gen(NCH - 1):
                    pass

    nc.compile()
    return nc


def make_in_maps(x, w_qkv, w_out):
    """Per-core numpy input dicts. Core c = batch (c // 4) x head quad (c % 4);
    quad g holds heads [g, 4+g, 8+g, 12+g] (slot order = slope tier)."""
    slopes = _slopes(NH)
    pos = np.arange(N, dtype=np.float32)
    kaug = np.stack([pos.astype(np.float16), np.ones(N, np.float16)])

    # xT packed chunk-major: xtp[p, 4096c + 512k + j] = x[b][512c + j, 128k + p]
    xtp16 = []
    for b in range(B):
        xT = np.ascontiguousarray(x[b].T).astype(np.float16)  # [1024, 2048]
        xtp16.append(np.ascontiguousarray(
            xT.reshape(KD, 128, NCH, CH).transpose(1, 2, 0, 3)
              .reshape(128, NCH * KD * CH)))

    def pack_w(w):  # [1024, 256] -> [128, 2048] with blocks of 256 per k
        return np.ascontiguousarray(
            w.reshape(KD, 128, 256).transpose(1, 0, 2).reshape(128, KD * 256))

    in_maps = []
    for core in range(8):
        b, g = core // 4, core % 4
        heads = [g, 4 + g, 8 + g, 12 + g]
        cols = np.concatenate([np.arange(h * D, (h + 1) * D) for h in heads])
        wq = pack_w((w_qkv[:, cols] * SCALE).astype(np.float16))
        wk = pack_w(w_qkv[:, DM + cols].astype(np.float16))
        wv = pack_w(w_qkv[:, 2 * DM + cols].astype(np.float16))
        wo = w_out[cols, :].astype(np.float16)  # [256, 1024]
        wo = np.ascontiguousarray(
            wo.reshape(2, 128, DM).transpose(1, 0, 2).reshape(128, 2 * DM))
        qa = np.empty((H, 2, N), np.float16)
        for s, h in enumerate(heads):
            s16 = np.float16(slopes[h])
            qa[s, 0, :] = s16
            qa[s, 1, :] = (-np.float32(s16) * pos).astype(np.float16)
        in_maps.append({
            "xtp": xtp16[b], "wqp": wq, "wkp": wk, "wvp": wv, "wop": wo,
            "qaug": qa, "kaug": kaug,
        })
    return in_maps


_NC_CACHE = []


def _get_nc():
    if not _NC_CACHE:
        _NC_CACHE.append(build_program())
    return _NC_CACHE[0]


def run_cores(in_maps, **kw):
    nc = _get_nc()
    return bass_utils.run_bass_kernel_spmd(nc, in_maps, core_ids=list(range(8)), **kw)


def kernel(x, w_qkv, w_out, b_out):
    x = np.asarray(x, np.float32)
    w_qkv = np.asarray(w_qkv, np.float32)
    w_out = np.asarray(w_out, np.float32)
    b_out = np.asarray(b_out, np.float32)
    res = run_cores(make_in_maps(x, w_qkv, w_out))
    out = np.zeros((B, N, DM), np.float32)
    for c in range(8):
        out[c // 4] += res.results[c]["out"].astype(np.float32)
    out += b_out[None, None, :]
    return out
